# revision 12
# baseline (speedup 1.0000x reference)
"""Causal multi-head self-attention with RoPE on 8 Trainium2 NeuronCores.

Problem: x[2, 2048, 1024] fp32, 16 heads, d_head=64, causal, RoPE(theta=1e4).
Sharding: core = b*4 + g  (b in {0,1} batch, g in {0..3} head-group of 4 heads).
Each core computes out_partial[2048, 1024] = attn(heads of g) @ wo[:, cols_g].T
in bf16; host sums the 4 partials per batch in fp32.

v2 pipeline (single TileContext, phases overlap via subtile deps):
  warmup) dummy matmuls on a memset tile warm the PE HAM clock while input
     DMAs land; a dummy exp preloads the ACT table.
  QKV) per seq-chunk of 512 (ascending): K proj + RoPE, V proj, Q proj +
     RoPE.  Projections are N=512 matmuls (V: N=256 into [seq,head,d+ones]
     layout); RoPE = sin-mul, pair-swap via perm matmul, cos-mul + add.
  D) per (q-chunk, head-pair): per k-tile: 2 scores matmuls (the two heads
     on partition halves 0:64/64:128 -> concurrent PE row groups), one exp
     ACTIVATE over [128, 2, 512-lo] (scale=1/8 fused, diagonal tiles
     free-sliced), causal mask via gpsimd affine_select on the partial
     block, 2 attn matmuls accumulating [65, 512] (ones row = softmax
     denominator).  Normalize: reciprocal_approx_fast on the denom row +
     gpsimd partition_broadcast + DVE mul.
  E) per q-chunk: out = attnT.T @ wo tiled 128x512, ko-outer for weight
     reuse, bf16 store.
"""

import os
import sys

sys.path.insert(0, "/opt/trn_rl_repo")

import ml_dtypes
import numpy as np

import concourse.bacc as bacc
import concourse.mybir as mybir
from concourse.tile import TileContext

B = 2
S = 2048
DM = 1024
H = 16
DH = 64
SC = 512  # seq chunk size
NQC = S // SC  # 4 chunks
P = 128
KO = DM // P  # 8 contraction subtiles for projections
SCALE = 1.0 / 8.0  # 1/sqrt(DH)
THETA = 10000.0

F32 = mybir.dt.float32
BF16 = mybir.dt.bfloat16

_CACHE = {}


def _build_nc():
    nc = bacc.Bacc("TRN2", enable_partition_id=False)
    Exp = mybir.ActivationFunctionType.Exp

    xT = nc.dram_tensor("xT", [DM, S], BF16, kind="ExternalInput")
    wq_t = nc.dram_tensor("wq_t", [DM, 256], BF16, kind="ExternalInput")
    wk_t = nc.dram_tensor("wk_t", [DM, 256], BF16, kind="ExternalInput")
    wv_t = nc.dram_tensor("wv_t", [DM, 256], BF16, kind="ExternalInput")
    wo_t = nc.dram_tensor("wo_t", [256, DM], BF16, kind="ExternalInput")
    cosT = nc.dram_tensor("cosT", [P, S], BF16, kind="ExternalInput")
    sinT = nc.dram_tensor("sinT", [P, S], BF16, kind="ExternalInput")
    perm = nc.dram_tensor("perm", [P, P], BF16, kind="ExternalInput")
    mask = nc.dram_tensor("mask128", [P, 2, P], BF16, kind="ExternalInput")
    outp = nc.dram_tensor("out_partial", [S, DM], BF16, kind="ExternalOutput")

    with TileContext(nc) as tc:
        with tc.tile_pool(name="persist", bufs=1) as persist, \
             tc.tile_pool(name="pp", bufs=2, space="PSUM") as pp, \
             tc.tile_pool(name="scp", bufs=2, space="PSUM") as scp, \
             tc.tile_pool(name="atp", bufs=1, space="PSUM") as atp, \
             tc.tile_pool(name="ptp", bufs=4) as ptp, \
             tc.tile_pool(name="t2p", bufs=2) as t2p, \
             tc.tile_pool(name="nrm", bufs=2) as nrm, \
             tc.tile_pool(name="ddr", bufs=4, space="DRAM") as ddr, \
             tc.tile_pool(name="otp", bufs=2) as otp:

            xT_sb = persist.tile([P, KO, S], BF16, tag="xT_sb")
            wq_sb = persist.tile([P, KO, 256], BF16, tag="wq_sb")
            wk_sb = persist.tile([P, KO, 256], BF16, tag="wk_sb")
            wv_sb = persist.tile([P, KO, 256], BF16, tag="wv_sb")
            wo_sb = persist.tile([P, 2, DM], BF16, tag="wo_sb")
            cos_sb = persist.tile([P, S], BF16, tag="cos_sb")
            sin_sb = persist.tile([P, S], BF16, tag="sin_sb")
            perm_sb = persist.tile([P, P], BF16, tag="perm_sb")
            mask_sb = persist.tile([P, 2, P], BF16, tag="mask_sb")
            q_rot = persist.tile([P, 2, S], BF16, tag="q_rot")
            k_rot = persist.tile([P, 2, S], BF16, tag="k_rot")
            v_sb = persist.tile([P, S // P, 4, 72], BF16, tag="v_sb")
            attnT = persist.tile([P, 2, S], BF16, tag="attnT")
            warm_sb = persist.tile([P, 640], BF16, tag="warm_sb")

            # ---- warmup: PE HAM + ACT table, while input DMAs land ------
            nc.vector.memset(warm_sb[:], 0.0)
            for _ in range(10):
                w_ps = pp.tile([P, SC], F32, tag="mm512", name="warm")
                nc.tensor.matmul(
                    w_ps[:], lhsT=warm_sb[:, 0:P], rhs=warm_sb[:, P:P + SC],
                    start=True, stop=True,
                )
            warm_pt = ptp.tile([P, 2, SC], BF16, tag="pt", name="warm_pt")
            nc.scalar.activation(
                out=warm_pt[:, 0, 0:64], in_=warm_sb[:, 0:64], func=Exp,
                scale=SCALE,
            )

            # ---- input DMAs (order = need order) ------------------------
            for t, d in ((wk_sb, wk_t), (wv_sb, wv_t), (wq_sb, wq_t)):
                nc.sync.dma_start(
                    t[:], d[:].rearrange("(ko p) m -> p ko m", p=P)
                )
            nc.sync.dma_start(cos_sb[:], cosT[:])
            nc.sync.dma_start(sin_sb[:], sinT[:])
            nc.sync.dma_start(perm_sb[:], perm[:])
            nc.sync.dma_start(mask_sb[:], mask[:])
            xT_ap = xT[:].rearrange("(ko p) s -> p ko s", p=P)
            for sc in range(NQC):
                cs = slice(sc * SC, (sc + 1) * SC)
                nc.sync.dma_start(xT_sb[:, :, cs], xT_ap[:, :, cs])
            nc.sync.dma_start(
                wo_sb[:], wo_t[:].rearrange("(ko p) m -> p ko m", p=P)
            )

            # ones column for the softmax-denominator trick
            nc.vector.memset(v_sb[:, :, :, 64:65], 1.0)

            # ---- QKV production, per seq chunk --------------------------
            def rope_proj(w_sb, dest, hp, sc):
                cs = slice(sc * SC, (sc + 1) * SC)
                a_ps = pp.tile([P, SC], F32, tag="mm512", name="a_ps")
                for ko in range(KO):
                    nc.tensor.matmul(
                        a_ps[:],
                        lhsT=w_sb[:, ko, hp * P:(hp + 1) * P],
                        rhs=xT_sb[:, ko, cs],
                        start=(ko == 0),
                        stop=(ko == KO - 1),
                    )
                t2 = t2p.tile([P, SC], BF16, tag="t2")
                nc.vector.tensor_mul(out=t2[:], in0=a_ps[:], in1=sin_sb[:, cs])
                b_ps = pp.tile([P, SC], F32, tag="mm512", name="b_ps")
                nc.tensor.matmul(
                    b_ps[:], lhsT=perm_sb[:], rhs=t2[:], start=True, stop=True
                )
                dsl = dest[:, hp, cs]
                nc.vector.tensor_mul(out=dsl, in0=a_ps[:], in1=cos_sb[:, cs])
                nc.vector.tensor_add(out=dsl, in0=dsl, in1=b_ps[:])

            for sc in range(NQC):
                for hp in range(2):
                    rope_proj(wk_sb, k_rot, hp, sc)
                for st in range(4 * sc, 4 * sc + 4):
                    v_ps = pp.tile([P, 256], F32, tag="mm512", name="v_ps")
                    for ko in range(KO):
                        nc.tensor.matmul(
                            v_ps[:],
                            lhsT=xT_sb[:, ko, st * P:(st + 1) * P],
                            rhs=wv_sb[:, ko, :],
                            start=(ko == 0),
                            stop=(ko == KO - 1),
                        )
                    nc.vector.tensor_copy(
                        out=v_sb[:, st, :, 0:64],
                        in_=v_ps[:].rearrange("p (h d) -> p h d", d=DH),
                    )
                for hp in range(2):
                    rope_proj(wq_sb, q_rot, hp, sc)

            # ---- attention (D) + output projection (E), per q chunk -----
            out_ap = outp[:].rearrange("(st p) m -> p st m", p=P)
            for qc in range(NQC):
                cs = slice(qc * SC, (qc + 1) * SC)
                nkt = 4 * qc + 4
                for hp in range(2):
                    at = [
                        atp.tile([65, SC], F32, tag=f"at{hh}",
                                 name=f"at{hh}")
                        for hh in range(2)
                    ]
                    for kt in range(nkt):
                        r = kt - 4 * qc
                        lo = 128 * r if r > 0 else 0
                        fsl = slice(lo, SC)
                        s2 = scp.tile([P, 2, SC], F32, tag="scores",
                                      name="s2")
                        for hh in range(2):
                            hs = slice(hh * 64, (hh + 1) * 64)
                            nc.tensor.matmul(
                                s2[:, hh, fsl],
                                lhsT=k_rot[hs, hp, kt * P:(kt + 1) * P],
                                rhs=q_rot[hs, hp, qc * SC + lo:(qc + 1) * SC],
                                start=True,
                                stop=True,
                            )
                        pt = ptp.tile([P, 2, SC], BF16, tag="pt", name="pt")
                        nc.scalar.activation(
                            out=pt[:, :, fsl], in_=s2[:, :, fsl], func=Exp,
                            scale=SCALE,
                        )
                        if r >= 0:
                            # causal mask on the partial diagonal block:
                            # keep col c (local) >= partition p
                            nc.vector.tensor_mul(
                                out=pt[:, :, lo:lo + P],
                                in0=pt[:, :, lo:lo + P],
                                in1=mask_sb[:],
                            )
                        for hh in range(2):
                            h = 2 * hp + hh
                            nc.tensor.matmul(
                                at[hh][:, fsl],
                                lhsT=v_sb[:, kt, h, 0:65],
                                rhs=pt[:, hh, fsl],
                                start=(kt == 0),
                                stop=(kt == nkt - 1),
                                skip_group_check=True,
                            )
                    # normalize: rows 0:64 attn, row 64 denominator
                    # (denominator broadcast across partitions via DRAM
                    # bounce -- the only partition-replication path that
                    # works on hardware)
                    for hh in range(2):
                        rt = nrm.tile([65, SC], F32, tag="rt")
                        nc.vector.tensor_copy(
                            out=rt[64:65, :], in_=at[hh][64:65, :]
                        )
                        dr = ddr.tile([1, SC], F32, tag="denr")
                        nc.sync.dma_start(dr[:], rt[64:65, :])
                        den_bc = nrm.tile([64, SC], F32, tag="den_bc")
                        nc.sync.dma_start(
                            den_bc[:], dr[:].partition_broadcast(64)
                        )
                        rbc = nrm.tile([64, SC], F32, tag="rbc")
                        nc.vector.reciprocal_approx_fast(
                            out=rbc[:], in_=den_bc[:]
                        )
                        if hh == 0:
                            nc.vector.tensor_mul(
                                out=attnT[0:64, hp, cs],
                                in0=at[hh][0:64, :],
                                in1=rbc[:],
                            )
                        else:
                            tmp = nrm.tile([64, SC], BF16, tag="tsh")
                            nc.vector.tensor_mul(
                                out=tmp[:], in0=at[hh][0:64, :], in1=rbc[:]
                            )
                            nc.sync.dma_start(attnT[64:128, hp, cs], tmp[:])
                # E for this q chunk
                for st in range(4 * qc, 4 * qc + 4):
                    o_ps = [
                        pp.tile([P, SC], F32, tag="mm512", name=f"o_ps{no}")
                        for no in range(2)
                    ]
                    for ko in range(2):
                        for no in range(2):
                            nc.tensor.matmul(
                                o_ps[no][:],
                                lhsT=attnT[:, ko, st * P:(st + 1) * P],
                                rhs=wo_sb[:, ko, no * SC:(no + 1) * SC],
                                start=(ko == 0),
                                stop=(ko == 1),
                            )
                    o_sb = otp.tile([P, DM], BF16, tag="o_sb")
                    for no in range(2):
                        nc.vector.tensor_copy(
                            out=o_sb[:, no * SC:(no + 1) * SC],
                            in_=o_ps[no][:],
                        )
                    nc.sync.dma_start(out_ap[:, st, :], o_sb[:])
    nc.compile()
    return nc


def _host_tables(token_positions):
    pos = np.asarray(token_positions).astype(np.float64)
    freq = 1.0 / (THETA ** (2.0 * np.arange(DH // 2, dtype=np.float64) / DH))
    ang = pos[:, None] * freq[None, :]  # [S, 32]
    cos_f = np.repeat(np.cos(ang), 2, axis=1)  # [S, 64]
    sin_f = np.repeat(np.sin(ang), 2, axis=1)
    cosT = np.ascontiguousarray(
        np.concatenate([cos_f.T, cos_f.T], axis=0)
    ).astype(ml_dtypes.bfloat16)  # [128, S]
    sinT = np.ascontiguousarray(
        np.concatenate([sin_f.T, sin_f.T], axis=0)
    ).astype(ml_dtypes.bfloat16)

    perm = np.zeros((P, P), dtype=ml_dtypes.bfloat16)
    for i in range(P // 2):
        perm[2 * i + 1, 2 * i] = -1.0
        perm[2 * i, 2 * i + 1] = 1.0

    tri = (np.arange(P)[None, :] >= np.arange(P)[:, None])
    mask128 = np.ascontiguousarray(
        np.broadcast_to(tri[:, None, :], (P, 2, P))
    ).astype(ml_dtypes.bfloat16)
    return cosT, sinT, perm, mask128


_LAST_RESULTS = None


def _bf16(a):
    return np.ascontiguousarray(a).astype(ml_dtypes.bfloat16)


def kernel(x, wq, wk, wv, wo, token_positions):
    global _LAST_RESULTS
    from concourse.bass_utils import run_bass_kernel_spmd

    if "nc" not in _CACHE:
        _CACHE["nc"] = _build_nc()
    nc = _CACHE["nc"]

    x = np.asarray(x, dtype=np.float32)
    wq = np.asarray(wq, dtype=np.float32)
    wk = np.asarray(wk, dtype=np.float32)
    wv = np.asarray(wv, dtype=np.float32)
    wo = np.asarray(wo, dtype=np.float32)
    cosT, sinT, perm, mask128 = _host_tables(token_positions)

    in_maps = []
    for b in range(B):
        xT_b = _bf16(x[b].T)  # [DM, S]
        for g in range(4):
            rows = slice(g * 256, (g + 1) * 256)
            in_maps.append(
                {
                    "xT": xT_b,
                    "wq_t": _bf16(wq[rows].T),
                    "wk_t": _bf16(wk[rows].T),
                    "wv_t": _bf16(wv[rows].T),
                    "wo_t": _bf16(wo[:, rows].T),
                    "cosT": cosT,
                    "sinT": sinT,
                    "perm": perm,
                    "mask128": mask128,
                }
            )

    res = run_bass_kernel_spmd(
        nc,
        in_maps,
        core_ids=list(range(8)),
        trace=bool(os.environ.get("BASS_TRACE")),
    )
    _LAST_RESULTS = res
    outs = res.results

    out = np.zeros((B, S, DM), dtype=np.float32)
    for b in range(B):
        for g in range(4):
            out[b] += outs[b * 4 + g]["out_partial"].astype(np.float32)
    return out


# revision 14
# speedup vs baseline: 1.1352x; 1.1352x over previous
"""Causal multi-head self-attention with RoPE on 8 Trainium2 NeuronCores.

Problem: x[2, 2048, 1024] fp32, 16 heads, d_head=64, causal, RoPE(theta=1e4).
Sharding: core = b*4 + g  (b in {0,1} batch, g in {0..3} head-group of 4 heads).
Each core computes out_partial[2048, 1024] = attn(heads of g) @ wo[:, cols_g].T
in bf16; host sums the 4 partials per batch in fp32.

v2 pipeline (single TileContext, phases overlap via subtile deps):
  warmup) dummy matmuls on a memset tile warm the PE HAM clock while input
     DMAs land; a dummy exp preloads the ACT table.
  QKV) per seq-chunk of 512 (ascending): K proj + RoPE, V proj, Q proj +
     RoPE.  Projections are N=512 matmuls (V: N=256 into [seq,head,d+ones]
     layout); RoPE = sin-mul, pair-swap via perm matmul, cos-mul + add.
  D) per (q-chunk, head-pair): per k-tile: 2 scores matmuls (the two heads
     on partition halves 0:64/64:128 -> concurrent PE row groups), one exp
     ACTIVATE over [128, 2, 512-lo] (scale=1/8 fused, diagonal tiles
     free-sliced), causal mask via gpsimd affine_select on the partial
     block, 2 attn matmuls accumulating [65, 512] (ones row = softmax
     denominator).  Normalize: reciprocal_approx_fast on the denom row +
     gpsimd partition_broadcast + DVE mul.
  E) per q-chunk: out = attnT.T @ wo tiled 128x512, ko-outer for weight
     reuse, bf16 store.
"""

import os
import sys

sys.path.insert(0, "/opt/trn_rl_repo")

import ml_dtypes
import numpy as np

import concourse.bacc as bacc
import concourse.mybir as mybir
from concourse.tile import TileContext

B = 2
S = 2048
DM = 1024
H = 16
DH = 64
SC = 512  # seq chunk size
NQC = S // SC  # 4 chunks
P = 128
KO = DM // P  # 8 contraction subtiles for projections
SCALE = 1.0 / 8.0  # 1/sqrt(DH)
THETA = 10000.0

F32 = mybir.dt.float32
BF16 = mybir.dt.bfloat16

_CACHE = {}


def _build_nc():
    nc = bacc.Bacc("TRN2", enable_partition_id=False)
    Exp = mybir.ActivationFunctionType.Exp

    xT = nc.dram_tensor("xT", [DM, S], BF16, kind="ExternalInput")
    wq_t = nc.dram_tensor("wq_t", [DM, 256], BF16, kind="ExternalInput")
    wk_t = nc.dram_tensor("wk_t", [DM, 256], BF16, kind="ExternalInput")
    wv_t = nc.dram_tensor("wv_t", [DM, 256], BF16, kind="ExternalInput")
    wo_t = nc.dram_tensor("wo_t", [256, DM], BF16, kind="ExternalInput")
    cosT = nc.dram_tensor("cosT", [P, S], BF16, kind="ExternalInput")
    sinT = nc.dram_tensor("sinT", [P, S], BF16, kind="ExternalInput")
    perm = nc.dram_tensor("perm", [P, P], BF16, kind="ExternalInput")
    mask = nc.dram_tensor("mask128", [P, 2, P], BF16, kind="ExternalInput")
    outp = nc.dram_tensor("out_partial", [S, DM], BF16, kind="ExternalOutput")

    with TileContext(nc) as tc:
        with tc.tile_pool(name="persist", bufs=1) as persist, \
             tc.tile_pool(name="pp", bufs=2, space="PSUM") as pp, \
             tc.tile_pool(name="scp", bufs=2, space="PSUM") as scp, \
             tc.tile_pool(name="atp", bufs=1, space="PSUM") as atp, \
             tc.tile_pool(name="ptp", bufs=4) as ptp, \
             tc.tile_pool(name="t2p", bufs=2) as t2p, \
             tc.tile_pool(name="nrm", bufs=2) as nrm, \
             tc.tile_pool(name="ddr", bufs=4, space="DRAM") as ddr, \
             tc.tile_pool(name="otp", bufs=2) as otp:

            xT_sb = persist.tile([P, KO, S], BF16, tag="xT_sb")
            wq_sb = persist.tile([P, KO, 256], BF16, tag="wq_sb")
            wk_sb = persist.tile([P, KO, 256], BF16, tag="wk_sb")
            wv_sb = persist.tile([P, KO, 256], BF16, tag="wv_sb")
            wo_sb = persist.tile([P, 2, DM], BF16, tag="wo_sb")
            cos_sb = persist.tile([P, S], BF16, tag="cos_sb")
            sin_sb = persist.tile([P, S], BF16, tag="sin_sb")
            perm_sb = persist.tile([P, P], BF16, tag="perm_sb")
            mask_sb = persist.tile([P, 2, P], BF16, tag="mask_sb")
            q_rot = persist.tile([P, 2, S], BF16, tag="q_rot")
            k_rot = persist.tile([P, 2, S], BF16, tag="k_rot")
            v_sb = persist.tile([P, S // P, 4, 72], BF16, tag="v_sb")
            attnT = persist.tile([P, 2, S], BF16, tag="attnT")
            warm_sb = persist.tile([P, 640], BF16, tag="warm_sb")

            # ---- warmup: PE HAM + ACT table, while input DMAs land ------
            nc.vector.memset(warm_sb[:], 0.0)
            for _ in range(10):
                w_ps = pp.tile([P, SC], F32, tag="mm512", name="warm")
                nc.tensor.matmul(
                    w_ps[:], lhsT=warm_sb[:, 0:P], rhs=warm_sb[:, P:P + SC],
                    start=True, stop=True,
                )
            warm_pt = ptp.tile([P, 2, SC], BF16, tag="pt", name="warm_pt")
            nc.scalar.activation(
                out=warm_pt[:, 0, 0:64], in_=warm_sb[:, 0:64], func=Exp,
                scale=SCALE,
            )

            # ---- input DMAs (order = need order) ------------------------
            xT_ap = xT[:].rearrange("(ko p) s -> p ko s", p=P)
            nc.sync.dma_start(
                wk_sb[:], wk_t[:].rearrange("(ko p) m -> p ko m", p=P)
            )
            nc.sync.dma_start(xT_sb[:, :, 0:SC], xT_ap[:, :, 0:SC])
            nc.sync.dma_start(cos_sb[:], cosT[:])
            nc.sync.dma_start(sin_sb[:], sinT[:])
            nc.sync.dma_start(perm_sb[:], perm[:])
            for t, d in ((wv_sb, wv_t), (wq_sb, wq_t)):
                nc.sync.dma_start(
                    t[:], d[:].rearrange("(ko p) m -> p ko m", p=P)
                )
            nc.sync.dma_start(mask_sb[:], mask[:])
            for sc in range(1, NQC):
                cs = slice(sc * SC, (sc + 1) * SC)
                nc.sync.dma_start(xT_sb[:, :, cs], xT_ap[:, :, cs])
            nc.sync.dma_start(
                wo_sb[:], wo_t[:].rearrange("(ko p) m -> p ko m", p=P)
            )

            # ones column for the softmax-denominator trick
            nc.vector.memset(v_sb[:, :, :, 64:65], 1.0)

            # ---- QKV production, per seq chunk --------------------------
            def rope_proj(w_sb, dest, hp, sc):
                cs = slice(sc * SC, (sc + 1) * SC)
                a_ps = pp.tile([P, SC], F32, tag="mm512", name="a_ps")
                for ko in range(KO):
                    nc.tensor.matmul(
                        a_ps[:],
                        lhsT=w_sb[:, ko, hp * P:(hp + 1) * P],
                        rhs=xT_sb[:, ko, cs],
                        start=(ko == 0),
                        stop=(ko == KO - 1),
                    )
                t2 = t2p.tile([P, SC], BF16, tag="t2")
                nc.vector.tensor_mul(out=t2[:], in0=a_ps[:], in1=sin_sb[:, cs])
                b_ps = pp.tile([P, SC], F32, tag="mm512", name="b_ps")
                nc.tensor.matmul(
                    b_ps[:], lhsT=perm_sb[:], rhs=t2[:], start=True, stop=True
                )
                dsl = dest[:, hp, cs]
                nc.vector.tensor_mul(out=dsl, in0=a_ps[:], in1=cos_sb[:, cs])
                nc.vector.tensor_add(out=dsl, in0=dsl, in1=b_ps[:])

            for sc in range(NQC):
                for hp in range(2):
                    rope_proj(wk_sb, k_rot, hp, sc)
                for st in range(4 * sc, 4 * sc + 4):
                    v_ps = pp.tile([P, 256], F32, tag="mm512", name="v_ps")
                    for ko in range(KO):
                        nc.tensor.matmul(
                            v_ps[:],
                            lhsT=xT_sb[:, ko, st * P:(st + 1) * P],
                            rhs=wv_sb[:, ko, :],
                            start=(ko == 0),
                            stop=(ko == KO - 1),
                        )
                    nc.vector.tensor_copy(
                        out=v_sb[:, st, :, 0:64],
                        in_=v_ps[:].rearrange("p (h d) -> p h d", d=DH),
                    )
                for hp in range(2):
                    rope_proj(wq_sb, q_rot, hp, sc)

            # ---- attention (D) + output projection (E), per q chunk -----
            out_ap = outp[:].rearrange("(st p) m -> p st m", p=P)
            for qc in range(NQC):
                cs = slice(qc * SC, (qc + 1) * SC)
                nkt = 4 * qc + 4
                for hp in range(2):
                    at = [
                        atp.tile([65, SC], F32, tag=f"at{hh}",
                                 name=f"at{hh}")
                        for hh in range(2)
                    ]
                    for kt in range(nkt):
                        r = kt - 4 * qc
                        lo = 128 * r if r > 0 else 0
                        fsl = slice(lo, SC)
                        s2 = scp.tile([P, 2, SC], F32, tag="scores",
                                      name="s2")
                        for hh in range(2):
                            hs = slice(hh * 64, (hh + 1) * 64)
                            nc.tensor.matmul(
                                s2[:, hh, fsl],
                                lhsT=k_rot[hs, hp, kt * P:(kt + 1) * P],
                                rhs=q_rot[hs, hp, qc * SC + lo:(qc + 1) * SC],
                                start=True,
                                stop=True,
                            )
                        pt = ptp.tile([P, 2, SC], BF16, tag="pt", name="pt")
                        nc.scalar.activation(
                            out=pt[:, :, fsl], in_=s2[:, :, fsl], func=Exp,
                            scale=SCALE,
                        )
                        if r >= 0:
                            # causal mask on the partial diagonal block:
                            # keep col c (local) >= partition p
                            nc.vector.tensor_mul(
                                out=pt[:, :, lo:lo + P],
                                in0=pt[:, :, lo:lo + P],
                                in1=mask_sb[:],
                            )
                        for hh in range(2):
                            h = 2 * hp + hh
                            nc.tensor.matmul(
                                at[hh][:, fsl],
                                lhsT=v_sb[:, kt, h, 0:65],
                                rhs=pt[:, hh, fsl],
                                start=(kt == 0),
                                stop=(kt == nkt - 1),
                                skip_group_check=True,
                            )
                    # normalize: rows 0:64 attn, row 64 denominator.
                    # Copy the whole psum tile to SBUF first so the at
                    # slot frees fast (next (hp,qc) attn matmuls don't
                    # stall on the DRAM-bounce denominator broadcast --
                    # the only partition-replication path that works on
                    # hardware).
                    a_cp = []
                    for hh in range(2):
                        cp = nrm.tile([65, SC], F32, tag=f"acp{hh}")
                        nc.vector.tensor_copy(out=cp[:], in_=at[hh][:])
                        a_cp.append(cp)
                    for hh in range(2):
                        dr = ddr.tile([1, SC], F32, tag="denr")
                        nc.sync.dma_start(dr[:], a_cp[hh][64:65, :])
                        den_bc = nrm.tile([64, SC], F32, tag="den_bc")
                        nc.sync.dma_start(
                            den_bc[:], dr[:].partition_broadcast(64)
                        )
                        rbc = nrm.tile([64, SC], F32, tag="rbc")
                        nc.vector.reciprocal_approx_fast(
                            out=rbc[:], in_=den_bc[:]
                        )
                        if hh == 0:
                            nc.vector.tensor_mul(
                                out=attnT[0:64, hp, cs],
                                in0=a_cp[hh][0:64, :],
                                in1=rbc[:],
                            )
                        else:
                            tmp = nrm.tile([64, SC], BF16, tag="tsh")
                            nc.vector.tensor_mul(
                                out=tmp[:], in0=a_cp[hh][0:64, :], in1=rbc[:]
                            )
                            nc.sync.dma_start(attnT[64:128, hp, cs], tmp[:])
                # E for this q chunk
                for st in range(4 * qc, 4 * qc + 4):
                    o_ps = [
                        pp.tile([P, SC], F32, tag="mm512", name=f"o_ps{no}")
                        for no in range(2)
                    ]
                    for ko in range(2):
                        for no in range(2):
                            nc.tensor.matmul(
                                o_ps[no][:],
                                lhsT=attnT[:, ko, st * P:(st + 1) * P],
                                rhs=wo_sb[:, ko, no * SC:(no + 1) * SC],
                                start=(ko == 0),
                                stop=(ko == 1),
                            )
                    o_sb = otp.tile([P, DM], BF16, tag="o_sb")
                    for no in range(2):
                        nc.vector.tensor_copy(
                            out=o_sb[:, no * SC:(no + 1) * SC],
                            in_=o_ps[no][:],
                        )
                    nc.sync.dma_start(out_ap[:, st, :], o_sb[:])
    nc.compile()
    return nc


def _host_tables(token_positions):
    pos = np.asarray(token_positions).astype(np.float64)
    freq = 1.0 / (THETA ** (2.0 * np.arange(DH // 2, dtype=np.float64) / DH))
    ang = pos[:, None] * freq[None, :]  # [S, 32]
    cos_f = np.repeat(np.cos(ang), 2, axis=1)  # [S, 64]
    sin_f = np.repeat(np.sin(ang), 2, axis=1)
    cosT = np.ascontiguousarray(
        np.concatenate([cos_f.T, cos_f.T], axis=0)
    ).astype(ml_dtypes.bfloat16)  # [128, S]
    sinT = np.ascontiguousarray(
        np.concatenate([sin_f.T, sin_f.T], axis=0)
    ).astype(ml_dtypes.bfloat16)

    perm = np.zeros((P, P), dtype=ml_dtypes.bfloat16)
    for i in range(P // 2):
        perm[2 * i + 1, 2 * i] = -1.0
        perm[2 * i, 2 * i + 1] = 1.0

    tri = (np.arange(P)[None, :] >= np.arange(P)[:, None])
    mask128 = np.ascontiguousarray(
        np.broadcast_to(tri[:, None, :], (P, 2, P))
    ).astype(ml_dtypes.bfloat16)
    return cosT, sinT, perm, mask128


_LAST_RESULTS = None


def _bf16(a):
    return np.ascontiguousarray(a).astype(ml_dtypes.bfloat16)


def kernel(x, wq, wk, wv, wo, token_positions):
    global _LAST_RESULTS
    from concourse.bass_utils import run_bass_kernel_spmd

    if "nc" not in _CACHE:
        _CACHE["nc"] = _build_nc()
    nc = _CACHE["nc"]

    x = np.asarray(x, dtype=np.float32)
    wq = np.asarray(wq, dtype=np.float32)
    wk = np.asarray(wk, dtype=np.float32)
    wv = np.asarray(wv, dtype=np.float32)
    wo = np.asarray(wo, dtype=np.float32)
    cosT, sinT, perm, mask128 = _host_tables(token_positions)

    in_maps = []
    for b in range(B):
        xT_b = _bf16(x[b].T)  # [DM, S]
        for g in range(4):
            rows = slice(g * 256, (g + 1) * 256)
            in_maps.append(
                {
                    "xT": xT_b,
                    "wq_t": _bf16(wq[rows].T),
                    "wk_t": _bf16(wk[rows].T),
                    "wv_t": _bf16(wv[rows].T),
                    "wo_t": _bf16(wo[:, rows].T),
                    "cosT": cosT,
                    "sinT": sinT,
                    "perm": perm,
                    "mask128": mask128,
                }
            )

    res = run_bass_kernel_spmd(
        nc,
        in_maps,
        core_ids=list(range(8)),
        trace=bool(os.environ.get("BASS_TRACE")),
    )
    _LAST_RESULTS = res
    outs = res.results

    out = np.zeros((B, S, DM), dtype=np.float32)
    for b in range(B):
        for g in range(4):
            out[b] += outs[b * 4 + g]["out_partial"].astype(np.float32)
    return out


# revision 23
# speedup vs baseline: 1.1440x; 1.0077x over previous
"""Causal multi-head self-attention with RoPE on 8 Trainium2 NeuronCores.

Problem: x[2, 2048, 1024] fp32, 16 heads, d_head=64, causal, RoPE(theta=1e4).
Sharding: core = b*4 + g  (b in {0,1} batch, g in {0..3} head-group of 4 heads).
Each core computes out_partial[2048, 1024] = attn(heads of g) @ wo[:, cols_g].T
in bf16; host sums the 4 partials per batch in fp32.

v2 pipeline (single TileContext, phases overlap via subtile deps):
  warmup) dummy matmuls on a memset tile warm the PE HAM clock while input
     DMAs land; a dummy exp preloads the ACT table.
  QKV) per seq-chunk of 512 (ascending): K proj + RoPE, V proj, Q proj +
     RoPE.  Projections are N=512 matmuls (V: N=256 into [seq,head,d+ones]
     layout); RoPE = sin-mul, pair-swap via perm matmul, cos-mul + add.
  D) per (q-chunk, head-pair): per k-tile: 2 scores matmuls (the two heads
     on partition halves 0:64/64:128 -> concurrent PE row groups), one exp
     ACTIVATE over [128, 2, 512-lo] (scale=1/8 fused, diagonal tiles
     free-sliced), causal mask via gpsimd affine_select on the partial
     block, 2 attn matmuls accumulating [65, 512] (ones row = softmax
     denominator).  Normalize: reciprocal_approx_fast on the denom row +
     gpsimd partition_broadcast + DVE mul.
  E) per q-chunk: out = attnT.T @ wo tiled 128x512, ko-outer for weight
     reuse, bf16 store.
"""

import os
import sys

sys.path.insert(0, "/opt/trn_rl_repo")

import ml_dtypes
import numpy as np

import concourse.bacc as bacc
import concourse.mybir as mybir
from concourse.tile import TileContext

B = 2
S = 2048
DM = 1024
H = 16
DH = 64
SC = 512  # seq chunk size
NQC = S // SC  # 4 chunks
P = 128
KO = DM // P  # 8 contraction subtiles for projections
SCALE = 1.0 / 8.0  # 1/sqrt(DH)
THETA = 10000.0

F32 = mybir.dt.float32
BF16 = mybir.dt.bfloat16

_CACHE = {}


def _build_nc():
    nc = bacc.Bacc("TRN2", enable_partition_id=False)
    Exp = mybir.ActivationFunctionType.Exp

    xT = nc.dram_tensor("xT", [DM, S], BF16, kind="ExternalInput")
    wq_t = nc.dram_tensor("wq_t", [DM, 256], BF16, kind="ExternalInput")
    wk_t = nc.dram_tensor("wk_t", [DM, 256], BF16, kind="ExternalInput")
    wv_t = nc.dram_tensor("wv_t", [DM, 256], BF16, kind="ExternalInput")
    wo_t = nc.dram_tensor("wo_t", [256, DM], BF16, kind="ExternalInput")
    cosT = nc.dram_tensor("cosT", [P, S], BF16, kind="ExternalInput")
    sinT = nc.dram_tensor("sinT", [P, S], BF16, kind="ExternalInput")
    perm = nc.dram_tensor("perm", [P, P], BF16, kind="ExternalInput")
    mask = nc.dram_tensor("mask128", [P, 2, P], BF16, kind="ExternalInput")
    outp = nc.dram_tensor("out_partial", [S, DM], BF16, kind="ExternalOutput")

    with TileContext(nc) as tc:
        with tc.tile_pool(name="persist", bufs=1) as persist, \
             tc.tile_pool(name="pp", bufs=2, space="PSUM") as pp, \
             tc.tile_pool(name="scp", bufs=2, space="PSUM") as scp, \
             tc.tile_pool(name="atp", bufs=1, space="PSUM") as atp, \
             tc.tile_pool(name="ptp", bufs=4) as ptp, \
             tc.tile_pool(name="t2p", bufs=2) as t2p, \
             tc.tile_pool(name="nrm", bufs=2) as nrm, \
             tc.tile_pool(name="ddr", bufs=4, space="DRAM") as ddr, \
             tc.tile_pool(name="otp", bufs=2) as otp:

            xT_sb = persist.tile([P, KO, S], BF16, tag="xT_sb")
            wq_sb = persist.tile([P, KO, 256], BF16, tag="wq_sb")
            wk_sb = persist.tile([P, KO, 256], BF16, tag="wk_sb")
            wv_sb = persist.tile([P, KO, 256], BF16, tag="wv_sb")
            wo_sb = persist.tile([P, 2, DM], BF16, tag="wo_sb")
            cos_sb = persist.tile([P, S], BF16, tag="cos_sb")
            sin_sb = persist.tile([P, S], BF16, tag="sin_sb")
            perm_sb = persist.tile([P, P], BF16, tag="perm_sb")
            mask_sb = persist.tile([P, 2, P], BF16, tag="mask_sb")
            ones_sb = persist.tile([65, 64], BF16, tag="ones_sb")
            q_rot = persist.tile([P, 2, S], BF16, tag="q_rot")
            k_rot = persist.tile([P, 2, S], BF16, tag="k_rot")
            v_sb = persist.tile([P, S // P, 4, 72], BF16, tag="v_sb")
            attnT = persist.tile([P, 2, S], BF16, tag="attnT")
            warm_sb = persist.tile([P, 640], BF16, tag="warm_sb")

            # ---- warmup: PE HAM + ACT table, while input DMAs land ------
            nc.vector.memset(warm_sb[:], 0.0)
            for _ in range(10):
                w_ps = pp.tile([P, SC], F32, tag="mm512", name="warm")
                nc.tensor.matmul(
                    w_ps[:], lhsT=warm_sb[:, 0:P], rhs=warm_sb[:, P:P + SC],
                    start=True, stop=True,
                )
            warm_pt = ptp.tile([P, 2, SC], BF16, tag="pt", name="warm_pt")
            nc.scalar.activation(
                out=warm_pt[:, 0, 0:64], in_=warm_sb[:, 0:64], func=Exp,
                scale=SCALE,
            )

            # ---- input DMAs (order = need order) ------------------------
            xT_ap = xT[:].rearrange("(ko p) s -> p ko s", p=P)
            wk_ap = wk_t[:].rearrange("(ko p) m -> p ko m", p=P)
            nc.sync.dma_start(wk_sb[:, 0:4, :], wk_ap[:, 0:4, :])
            nc.sync.dma_start(wk_sb[:, 4:8, :], wk_ap[:, 4:8, :])
            for ko2 in range(4):
                nc.sync.dma_start(
                    xT_sb[:, 2 * ko2:2 * ko2 + 2, 0:SC],
                    xT_ap[:, 2 * ko2:2 * ko2 + 2, 0:SC],
                )
            nc.sync.dma_start(cos_sb[:], cosT[:])
            nc.sync.dma_start(sin_sb[:], sinT[:])
            nc.sync.dma_start(perm_sb[:], perm[:])
            for t, d in ((wv_sb, wv_t), (wq_sb, wq_t)):
                nc.sync.dma_start(
                    t[:], d[:].rearrange("(ko p) m -> p ko m", p=P)
                )
            nc.sync.dma_start(mask_sb[:], mask[:])
            for sc in range(1, NQC):
                cs = slice(sc * SC, (sc + 1) * SC)
                nc.sync.dma_start(xT_sb[:, :, cs], xT_ap[:, :, cs])
            nc.sync.dma_start(
                wo_sb[:], wo_t[:].rearrange("(ko p) m -> p ko m", p=P)
            )

            # ones column for the softmax-denominator trick
            nc.vector.memset(v_sb[:, :, :, 64:65], 1.0)
            nc.vector.memset(ones_sb[:], 1.0)

            # ---- QKV production, per seq chunk --------------------------
            def rope_proj(w_sb, dest, hp, sc):
                cs = slice(sc * SC, (sc + 1) * SC)
                a_ps = pp.tile([P, SC], F32, tag="mm512", name="a_ps")
                for ko in range(KO):
                    nc.tensor.matmul(
                        a_ps[:],
                        lhsT=w_sb[:, ko, hp * P:(hp + 1) * P],
                        rhs=xT_sb[:, ko, cs],
                        start=(ko == 0),
                        stop=(ko == KO - 1),
                    )
                t2 = t2p.tile([P, SC], BF16, tag="t2")
                nc.vector.tensor_mul(out=t2[:], in0=a_ps[:], in1=sin_sb[:, cs])
                b_ps = pp.tile([P, SC], F32, tag="mm512", name="b_ps")
                nc.tensor.matmul(
                    b_ps[:], lhsT=perm_sb[:], rhs=t2[:], start=True, stop=True
                )
                dsl = dest[:, hp, cs]
                nc.vector.tensor_mul(out=dsl, in0=a_ps[:], in1=cos_sb[:, cs])
                nc.vector.tensor_add(out=dsl, in0=dsl, in1=b_ps[:])

            for sc in range(NQC):
                for hp in range(2):
                    rope_proj(wk_sb, k_rot, hp, sc)
                for st in range(4 * sc, 4 * sc + 4):
                    v_ps = pp.tile([P, 256], F32, tag="mm512", name="v_ps")
                    for ko in range(KO):
                        nc.tensor.matmul(
                            v_ps[:],
                            lhsT=xT_sb[:, ko, st * P:(st + 1) * P],
                            rhs=wv_sb[:, ko, :],
                            start=(ko == 0),
                            stop=(ko == KO - 1),
                        )
                    nc.vector.tensor_copy(
                        out=v_sb[:, st, :, 0:64],
                        in_=v_ps[:].rearrange("p (h d) -> p h d", d=DH),
                    )
                for hp in range(2):
                    rope_proj(wq_sb, q_rot, hp, sc)

            # ---- attention (D) + output projection (E), per q chunk -----
            out_ap = outp[:].rearrange("(st p) m -> p st m", p=P)
            for qc in range(NQC):
                cs = slice(qc * SC, (qc + 1) * SC)
                nkt = 4 * qc + 4
                for hp in range(2):
                    at = [
                        atp.tile([65, SC], F32, tag=f"at{hh}",
                                 name=f"at{hh}")
                        for hh in range(2)
                    ]
                    for kt in range(nkt):
                        r = kt - 4 * qc
                        lo = 128 * r if r > 0 else 0
                        fsl = slice(lo, SC)
                        s2 = scp.tile([P, 2, SC], F32, tag="scores",
                                      name="s2")
                        for hh in range(2):
                            hs = slice(hh * 64, (hh + 1) * 64)
                            nc.tensor.matmul(
                                s2[:, hh, fsl],
                                lhsT=k_rot[hs, hp, kt * P:(kt + 1) * P],
                                rhs=q_rot[hs, hp, qc * SC + lo:(qc + 1) * SC],
                                start=True,
                                stop=True,
                            )
                        pt = ptp.tile([P, 2, SC], BF16, tag="pt", name="pt")
                        nc.scalar.activation(
                            out=pt[:, :, fsl], in_=s2[:, :, fsl], func=Exp,
                            scale=SCALE,
                        )
                        if r >= 0:
                            # causal mask on the partial diagonal block:
                            # keep col c (local) >= partition p
                            nc.vector.tensor_mul(
                                out=pt[:, :, lo:lo + P],
                                in0=pt[:, :, lo:lo + P],
                                in1=mask_sb[:],
                            )
                        for hh in range(2):
                            h = 2 * hp + hh
                            nc.tensor.matmul(
                                at[hh][:, fsl],
                                lhsT=v_sb[:, kt, h, 0:65],
                                rhs=pt[:, hh, fsl],
                                start=(kt == 0),
                                stop=(kt == nkt - 1),
                                skip_group_check=True,
                            )
                    # normalize: rows 0:64 attn, row 64 denominator.
                    # Copy the whole psum tile to SBUF first so the at
                    # slot frees fast (next (hp,qc) attn matmuls don't
                    # stall on the DRAM-bounce denominator broadcast --
                    # the only partition-replication path that works on
                    # hardware).
                    a_cp = []
                    for hh in range(2):
                        cp = nrm.tile([65, SC], F32, tag=f"acp{hh}")
                        nc.vector.tensor_copy(out=cp[:], in_=at[hh][:])
                        a_cp.append(cp)
                    for hh in range(2):
                        # denominator broadcast across partitions via DRAM
                        # bounce -- the only partition-replication path
                        # that is reliable on hardware
                        dr = ddr.tile([1, SC], F32, tag="denr")
                        nc.sync.dma_start(dr[:], a_cp[hh][64:65, :])
                        den_bc = nrm.tile([64, SC], F32, tag="den_bc")
                        nc.sync.dma_start(
                            den_bc[:], dr[:].partition_broadcast(64)
                        )
                        rbc = nrm.tile([64, SC], F32, tag="rbc")
                        nc.vector.reciprocal_approx_fast(
                            out=rbc[:], in_=den_bc[:]
                        )
                        if hh == 0:
                            nc.vector.tensor_mul(
                                out=attnT[0:64, hp, cs],
                                in0=a_cp[hh][0:64, :],
                                in1=rbc[:],
                            )
                        else:
                            tmp = nrm.tile([64, SC], BF16, tag="tsh")
                            nc.vector.tensor_mul(
                                out=tmp[:], in0=a_cp[hh][0:64, :],
                                in1=rbc[:],
                            )
                            nc.sync.dma_start(attnT[64:128, hp, cs], tmp[:])
                # E for this q chunk
                for st in range(4 * qc, 4 * qc + 4):
                    o_ps = [
                        pp.tile([P, SC], F32, tag="mm512", name=f"o_ps{no}")
                        for no in range(2)
                    ]
                    for ko in range(2):
                        for no in range(2):
                            nc.tensor.matmul(
                                o_ps[no][:],
                                lhsT=attnT[:, ko, st * P:(st + 1) * P],
                                rhs=wo_sb[:, ko, no * SC:(no + 1) * SC],
                                start=(ko == 0),
                                stop=(ko == 1),
                            )
                    o_sb = otp.tile([P, DM], BF16, tag="o_sb")
                    for no in range(2):
                        nc.vector.tensor_copy(
                            out=o_sb[:, no * SC:(no + 1) * SC],
                            in_=o_ps[no][:],
                        )
                    nc.sync.dma_start(out_ap[:, st, :], o_sb[:])
    nc.compile()
    return nc


def _host_tables(token_positions):
    pos = np.asarray(token_positions).astype(np.float64)
    freq = 1.0 / (THETA ** (2.0 * np.arange(DH // 2, dtype=np.float64) / DH))
    ang = pos[:, None] * freq[None, :]  # [S, 32]
    cos_f = np.repeat(np.cos(ang), 2, axis=1)  # [S, 64]
    sin_f = np.repeat(np.sin(ang), 2, axis=1)
    cosT = np.ascontiguousarray(
        np.concatenate([cos_f.T, cos_f.T], axis=0)
    ).astype(ml_dtypes.bfloat16)  # [128, S]
    sinT = np.ascontiguousarray(
        np.concatenate([sin_f.T, sin_f.T], axis=0)
    ).astype(ml_dtypes.bfloat16)

    perm = np.zeros((P, P), dtype=ml_dtypes.bfloat16)
    for i in range(P // 2):
        perm[2 * i + 1, 2 * i] = -1.0
        perm[2 * i, 2 * i + 1] = 1.0

    tri = (np.arange(P)[None, :] >= np.arange(P)[:, None])
    mask128 = np.ascontiguousarray(
        np.broadcast_to(tri[:, None, :], (P, 2, P))
    ).astype(ml_dtypes.bfloat16)
    return cosT, sinT, perm, mask128


_LAST_RESULTS = None


def _bf16(a):
    return np.ascontiguousarray(a).astype(ml_dtypes.bfloat16)


def kernel(x, wq, wk, wv, wo, token_positions):
    global _LAST_RESULTS
    from concourse.bass_utils import run_bass_kernel_spmd

    if "nc" not in _CACHE:
        _CACHE["nc"] = _build_nc()
    nc = _CACHE["nc"]

    x = np.asarray(x, dtype=np.float32)
    wq = np.asarray(wq, dtype=np.float32)
    wk = np.asarray(wk, dtype=np.float32)
    wv = np.asarray(wv, dtype=np.float32)
    wo = np.asarray(wo, dtype=np.float32)
    cosT, sinT, perm, mask128 = _host_tables(token_positions)

    in_maps = []
    for b in range(B):
        xT_b = _bf16(x[b].T)  # [DM, S]
        for g in range(4):
            rows = slice(g * 256, (g + 1) * 256)
            in_maps.append(
                {
                    "xT": xT_b,
                    "wq_t": _bf16(wq[rows].T),
                    "wk_t": _bf16(wk[rows].T),
                    "wv_t": _bf16(wv[rows].T),
                    "wo_t": _bf16(wo[:, rows].T),
                    "cosT": cosT,
                    "sinT": sinT,
                    "perm": perm,
                    "mask128": mask128,
                }
            )

    res = run_bass_kernel_spmd(
        nc,
        in_maps,
        core_ids=list(range(8)),
        trace=bool(os.environ.get("BASS_TRACE")),
    )
    _LAST_RESULTS = res
    outs = res.results

    out = np.zeros((B, S, DM), dtype=np.float32)
    for b in range(B):
        for g in range(4):
            out[b] += outs[b * 4 + g]["out_partial"].astype(np.float32)
    return out
